# revision 12
# baseline (speedup 1.0000x reference)
"""Trainium2 Bass kernel for nn_EquiPINE (pooling).

Math (per branch):
    W_g = (U @ A).reshape(M, L); w = W_g @ P  -> [M]
    g = sigmoid(x[...,None] * w + V)          -> [B, N, D, M]
    out = sum_n max_d g                       -> [B, M]

Key restructuring: sigmoid is monotonic, so
    max_d sigmoid(x*w + V) = sigmoid(max_d(x*w) + V)
and max_d(x[b,n,d]*w[m]) = w_pos[m]*xmax[b,n] + w_neg[m]*xmin[b,n]
(with w_pos = max(w,0), w_neg = min(w,0)), i.e. a K=2 matmul between
[2, M] weights and [2, N] rows (xmax; xmin).  The [B,N,D,M] intermediate
never exists; per (batch, branch) work is one K=2 matmul -> PSUM [128, N]
followed by one ACT sigmoid(+bias V, accumulate-over-free) -> z column.

Sharding: data-parallel over batch; 8 batches per core on 8 cores.
Params are tiny and replicated; host precomputes w_pos/w_neg, W_h^T,
C_w^T (numpy, microscopic cost) so the device kernel needs no transposes.
"""

import numpy as np

import concourse.bass as bass
import concourse.bacc as bacc
import concourse.tile as tile
from concourse import mybir
from concourse.bass_utils import run_bass_kernel_spmd

NCORES = 8
B = 64
B_LOC = B // NCORES  # 8 batches per core
N = 1024
D = 16
M = 128
L = 32
H = 256
O = 128

F32 = mybir.dt.float32
BF16 = mybir.dt.bfloat16
AF = mybir.ActivationFunctionType
ALU = mybir.AluOpType
AX = mybir.AxisListType


def _emit(tc, io):
    nc = tc.nc
    with (
        tc.tile_pool(name="const", bufs=1) as cpool,
        tc.tile_pool(name="xp", bufs=1) as xpool,
        tc.tile_pool(name="stat", bufs=1) as spool,
        tc.tile_pool(name="sig", bufs=2) as sigpool,
        tc.tile_pool(name="ps", bufs=2, space="PSUM") as pspool,
        tc.tile_pool(name="psmlp", bufs=2, space="PSUM") as mlppool,
    ):
        # ---- params to SBUF ----
        lhs_t = []
        v_t = []
        for br in range(2):
            lt = cpool.tile([8, M], BF16, tag=f"lhs{br}")
            nc.sync.dma_start(lt[:], io[f"lhs{br}"])
            lhs_t.append(lt)
            vt = cpool.tile([M, 1], F32, tag=f"v{br}")
            nc.sync.dma_start(vt[:], io[f"v{br}"])
            v_t.append(vt)
        wh_t = []
        cw_t = []
        for k in range(2):
            wt = cpool.tile([128, H], F32, tag=f"wh{k}")
            nc.sync.dma_start(wt[:], io["wht"][k * 128 : (k + 1) * 128, :])
            wh_t.append(wt)
            ct = cpool.tile([128, O], F32, tag=f"cw{k}")
            nc.sync.dma_start(ct[:], io["cwt"][k * 128 : (k + 1) * 128, :])
            cw_t.append(ct)
        cb_t = cpool.tile([1, O], F32, tag="cb")
        nc.sync.dma_start(cb_t[:], io["cb"])
        ones_t = cpool.tile([1, B_LOC], F32, tag="ones")
        nc.vector.memset(ones_t[:], 1.0)

        # z columns per branch: [M, B_LOC]
        z_t = [
            cpool.tile([M, B_LOC], F32, tag=f"z{br}", name=f"z{br}")
            for br in range(2)
        ]

        # ---- per x-tensor: load, d-reduce (max/min), shuffle to row layout ----
        # x shard viewed as [128, 1024]: partition p = b*16 + n//64,
        # free f = (n%64)*16 + d.
        r_t = []
        for xi in range(2):
            xt = xpool.tile([128, N], F32, tag=f"x{xi}")
            nc.sync.dma_start(xt[:], io[f"x{xi}"])
            x3 = xt[:].rearrange("p (c d) -> p c d", d=D)
            xmax = spool.tile([128, 64], F32, tag=f"xmax{xi}")
            xmin = spool.tile([128, 64], F32, tag=f"xmin{xi}")
            nc.vector.tensor_reduce(xmax[:], x3, axis=AX.X, op=ALU.max)
            nc.vector.tensor_reduce(xmin[:], x3, axis=AX.X, op=ALU.min)
            # bf16 hi/lo split of xmax/xmin (bf16x2 keeps ~fp32 precision
            # while the PE runs at the 1 cycle/row bf16 rate)
            hilo = []
            for sname, stat in (("xmax", xmax), ("xmin", xmin)):
                hi = spool.tile([128, 64], BF16, tag=f"{sname}h{xi}", name="hi")
                nc.vector.tensor_copy(hi[:], stat[:])
                hi32 = spool.tile([128, 64], F32, tag=f"{sname}h32{xi}", name="hi32")
                nc.vector.tensor_copy(hi32[:], hi[:])
                d32 = spool.tile([128, 64], F32, tag=f"{sname}d32{xi}", name="d32")
                nc.vector.tensor_tensor(d32[:], stat[:], hi32[:], op=ALU.subtract)
                lo = spool.tile([128, 64], BF16, tag=f"{sname}l{xi}", name="lo")
                nc.vector.tensor_copy(lo[:], d32[:])
                hilo += [hi, lo]
            # R rows (K=8): xmh xmh xml xml xnh xnh xnl xnl, free index
            # b*1024 + c*64 + j  <->  (batch b, n = c*64 + j)
            rt = spool.tile([8, B_LOC * N], BF16, tag=f"r{xi}")
            for row, src in enumerate([0, 0, 1, 1, 2, 2, 3, 3]):
                nc.sync.dma_start(
                    rt[row : row + 1, :].rearrange(
                        "p (b c j) -> p b c j", c=16, j=64
                    ),
                    hilo[src][:],
                )
            r_t.append(rt)

        # ---- branch core: K=2 matmul + fused sigmoid/bias/accumulate ----
        for br in range(2):
            lt = lhs_t[br][:]
            for b in range(B_LOC):
                ps = pspool.tile([M, N], F32, tag="s")
                rhs = r_t[br][:, b * N : (b + 1) * N]
                nc.tensor.matmul(
                    ps[:, 0:512], lt, rhs[:, 0:512], start=True, stop=True
                )
                nc.tensor.matmul(
                    ps[:, 512:1024], lt, rhs[:, 512:1024], start=True, stop=True
                )
                sg = sigpool.tile([M, N], F32, tag="sg")
                nc.scalar.activation(
                    sg[:],
                    ps[:],
                    AF.Sigmoid,
                    bias=v_t[br][:],
                    accum_out=z_t[br][:, b : b + 1],
                )

        # ---- MLP head: h = sigmoid(W_h @ z); y = C_w @ h + C_b ----
        h_t = []
        for hh in range(2):
            hp = mlppool.tile([128, B_LOC], F32, tag="mlp")
            nc.tensor.matmul(
                hp[:],
                wh_t[0][:, hh * 128 : (hh + 1) * 128],
                z_t[0][:],
                start=True,
                stop=False,
            )
            nc.tensor.matmul(
                hp[:],
                wh_t[1][:, hh * 128 : (hh + 1) * 128],
                z_t[1][:],
                start=False,
                stop=True,
            )
            hs = spool.tile([128, B_LOC], F32, tag=f"hs{hh}")
            nc.scalar.activation(hs[:], hp[:], AF.Sigmoid)
            h_t.append(hs)

        yp = mlppool.tile([B_LOC, O], F32, tag="mlp")
        nc.tensor.matmul(yp[:], h_t[0][:], cw_t[0][:], start=True, stop=False)
        nc.tensor.matmul(yp[:], h_t[1][:], cw_t[1][:], start=False, stop=False)
        nc.tensor.matmul(yp[:], ones_t[:], cb_t[:], start=False, stop=True)
        y_sb = spool.tile([B_LOC, O], F32, tag="ysb")
        nc.vector.tensor_copy(y_sb[:], yp[:])
        nc.sync.dma_start(io["y"], y_sb[:])


_CACHED = None


def _build():
    global _CACHED
    if _CACHED is not None:
        return _CACHED
    nc = bacc.Bacc(
        "TRN2", target_bir_lowering=False, debug=False, num_devices=NCORES
    )
    io = {}
    io["x0"] = nc.dram_tensor("x0", [128, N], F32, kind="ExternalInput").ap()
    io["x1"] = nc.dram_tensor("x1", [128, N], F32, kind="ExternalInput").ap()
    for br in range(2):
        io[f"lhs{br}"] = nc.dram_tensor(
            f"lhs{br}", [8, M], BF16, kind="ExternalInput"
        ).ap()
        io[f"v{br}"] = nc.dram_tensor(
            f"v{br}", [M, 1], F32, kind="ExternalInput"
        ).ap()
    io["wht"] = nc.dram_tensor("wht", [2 * M, H], F32, kind="ExternalInput").ap()
    io["cwt"] = nc.dram_tensor("cwt", [H, O], F32, kind="ExternalInput").ap()
    io["cb"] = nc.dram_tensor("cb", [1, O], F32, kind="ExternalInput").ap()
    io["y"] = nc.dram_tensor("y", [B_LOC, O], F32, kind="ExternalOutput").ap()

    with tile.TileContext(nc) as tc:
        _emit(tc, io)
    nc.compile()
    _CACHED = nc
    return nc


def _prep_params(inputs):
    import ml_dtypes

    f = np.float32
    bf = ml_dtypes.bfloat16

    def branch_lhs(P, U, A):
        W_g = (U @ A).reshape(M, L).astype(np.float64)
        w = (W_g @ P.astype(np.float64))[:, 0]
        rows = []
        for part in (np.maximum(w, 0.0), np.minimum(w, 0.0)):
            hi = part.astype(f).astype(bf)
            lo = (part.astype(f) - hi.astype(f)).astype(bf)
            rows += [hi, lo, hi, lo]
        # rows: wph wpl wph wpl wnh wnl wnh wnl -> pair with R's
        # xmh xmh xml xml xnh xnh xnl xnl
        return np.stack(rows).astype(bf)

    out = {
        "lhs0": branch_lhs(inputs["P0"], inputs["U0"], inputs["A0"]),
        "lhs1": branch_lhs(inputs["P1"], inputs["U1"], inputs["A1"]),
        "v0": np.ascontiguousarray(inputs["V0"].reshape(M, 1), dtype=f),
        "v1": np.ascontiguousarray(inputs["V1"].reshape(M, 1), dtype=f),
        "wht": np.ascontiguousarray(inputs["W_h"].T, dtype=f),
        "cwt": np.ascontiguousarray(inputs["C_w"].T, dtype=f),
        "cb": np.ascontiguousarray(inputs["C_b"].reshape(1, O), dtype=f),
    }
    return out


def run(inputs, trace=False, **kw):
    nc = _build()
    params = _prep_params(inputs)
    x0 = np.ascontiguousarray(inputs["x0"], dtype=np.float32)
    x1 = np.ascontiguousarray(inputs["x1"], dtype=np.float32)
    in_maps = []
    for c in range(NCORES):
        m = dict(params)
        m["x0"] = x0[c * B_LOC : (c + 1) * B_LOC].reshape(128, N)
        m["x1"] = x1[c * B_LOC : (c + 1) * B_LOC].reshape(128, N)
        in_maps.append(m)
    res = run_bass_kernel_spmd(nc, in_maps, list(range(NCORES)), trace=trace, **kw)
    y = np.concatenate([res.results[c]["y"] for c in range(NCORES)], axis=0)
    return y, res


def kernel(**inputs):
    y, _ = run(inputs, trace=False)
    return y


# revision 13
# speedup vs baseline: 1.0867x; 1.0867x over previous
"""Trainium2 Bass kernel for nn_EquiPINE (pooling).

Math (per branch):
    W_g = (U @ A).reshape(M, L); w = W_g @ P  -> [M]
    g = sigmoid(x[...,None] * w + V)          -> [B, N, D, M]
    out = sum_n max_d g                       -> [B, M]

Key restructuring: sigmoid is monotonic, so
    max_d sigmoid(x*w + V) = sigmoid(max_d(x*w) + V)
and max_d(x[b,n,d]*w[m]) = w_pos[m]*xmax[b,n] + w_neg[m]*xmin[b,n]
(with w_pos = max(w,0), w_neg = min(w,0)).  With xmax/xmin and the w
vectors each split into bf16 hi+lo pairs (bf16x2 ~= fp32 precision at
the PE's 1 cycle/row bf16 rate), the whole [B,N,D,M] intermediate
collapses into one K=8 matmul -> PSUM [128, N] per (batch, branch),
followed by one ACT sigmoid (+per-partition bias V, accumulate-over-
free) producing the pooled z column directly.

Sharding: data-parallel over batch; 8 batches per core on 8 cores.
Params are tiny and replicated; host precomputes w hi/lo rows, W_h^T,
C_w^T and packs all f32 params into one DMA (HWDGE issue costs ~625ns
of engine time per dma_start, so DMA count dominates the prep phase).
"""

import numpy as np

import concourse.bass as bass
import concourse.bacc as bacc
import concourse.tile as tile
from concourse import mybir
from concourse.bass_utils import run_bass_kernel_spmd

NCORES = 8
B = 64
B_LOC = B // NCORES  # 8 batches per core
N = 1024
D = 16
M = 128
L = 32
H = 256
O = 128

F32 = mybir.dt.float32
BF16 = mybir.dt.bfloat16
AF = mybir.ActivationFunctionType
ALU = mybir.AluOpType
AX = mybir.AxisListType

# packed f32 param column layout: wht0 wht1 cw0 cw1 v0 v1
PK_WHT0 = 0
PK_WHT1 = 256
PK_CW0 = 512
PK_CW1 = 640
PK_V0 = 768
PK_V1 = 769
PK_COLS = 770


def _emit(tc, io):
    nc = tc.nc
    with (
        tc.tile_pool(name="const", bufs=1) as cpool,
        tc.tile_pool(name="xp", bufs=1) as xpool,
        tc.tile_pool(name="stat", bufs=1) as spool,
        tc.tile_pool(name="sig", bufs=2) as sigpool,
        tc.tile_pool(name="ps", bufs=3, space="PSUM") as pspool,
        tc.tile_pool(name="psmlp", bufs=1, space="PSUM") as mlppool,
    ):
        # ---- x loads first (sync queue), params on the scalar DGE ----
        xt = []
        for xi in range(2):
            t = xpool.tile([128, N], F32, tag=f"x{xi}", name=f"xt{xi}")
            nc.sync.dma_start(t[:], io[f"x{xi}"])
            xt.append(t)
        pack = cpool.tile([128, PK_COLS], F32, tag="pack")
        nc.scalar.dma_start(pack[:], io["pack"])
        lhs = cpool.tile([8, 2 * M], BF16, tag="lhs")
        nc.scalar.dma_start(lhs[:], io["lhs"])
        cb_t = cpool.tile([1, O], F32, tag="cb")
        nc.scalar.dma_start(cb_t[:], io["cb"])
        ones_t = cpool.tile([1, B_LOC], F32, tag="ones")
        nc.gpsimd.memset(ones_t[:], 1.0)

        # z columns per branch: [M, B_LOC]
        z_t = [
            cpool.tile([M, B_LOC], F32, tag=f"z{br}", name=f"z{br}")
            for br in range(2)
        ]

        # ---- per x-tensor: d-reduce (max/min), bf16 hi/lo, row shuffle ----
        # x shard viewed as [128, 1024]: partition p = b*16 + n//64,
        # free f = (n%64)*16 + d.
        dma_eng = [nc.sync, nc.scalar]
        r_t = []
        for xi in range(2):
            x3 = xt[xi][:].rearrange("p (c d) -> p c d", d=D)
            xmax = spool.tile([128, 64], F32, tag=f"xmax{xi}", name="xmax")
            xmin = spool.tile([128, 64], F32, tag=f"xmin{xi}", name="xmin")
            nc.vector.tensor_reduce(xmax[:], x3, axis=AX.X, op=ALU.max)
            nc.vector.tensor_reduce(xmin[:], x3, axis=AX.X, op=ALU.min)
            # comb columns: xmax_hi | xmax_lo | xmin_hi | xmin_lo (bf16)
            comb = spool.tile([128, 256], BF16, tag=f"comb{xi}", name="comb")
            for si, stat in enumerate((xmax, xmin)):
                hic = comb[:, si * 128 : si * 128 + 64]
                nc.vector.tensor_copy(hic, stat[:])
                hi32 = spool.tile([128, 64], F32, tag=f"hi32_{xi}{si}", name="hi32")
                nc.vector.tensor_copy(hi32[:], hic)
                d32 = spool.tile([128, 64], F32, tag=f"d32_{xi}{si}", name="d32")
                nc.vector.tensor_tensor(d32[:], stat[:], hi32[:], op=ALU.subtract)
                nc.vector.tensor_copy(comb[:, si * 128 + 64 : si * 128 + 128], d32[:])
            # R rows 0..3 = xmh xml xnh xnl; free = b*1024 + c*64 + j
            # (n = c*64 + j); rows 4..7 duplicate rows 0..3.
            rt = spool.tile([8, B_LOC * N], BF16, tag=f"r{xi}", name="rt")
            eng = dma_eng[xi]
            for row in range(4):
                eng.dma_start(
                    rt[row : row + 1, :].rearrange(
                        "p (b c j) -> p b c j", c=16, j=64
                    ),
                    comb[:, row * 64 : (row + 1) * 64],
                )
            eng.dma_start(rt[4:8, :], rt[0:4, :])
            r_t.append(rt)

        # ---- branch core: K=8 bf16x2 matmul + fused sigmoid/bias/accum ----
        # lhs rows: wph wph wnh wnh wpl wpl wnl wnl pair with
        # rt rows:  xmh xml xnh xnl xmh xml xnh xnl
        for br in range(2):
            lt = lhs[:, br * M : (br + 1) * M]
            vt = pack[:, PK_V0 + br : PK_V0 + br + 1]
            for b in range(B_LOC):
                ps = pspool.tile([M, N], F32, tag="s", name="ps")
                rhs = r_t[br][:, b * N : (b + 1) * N]
                nc.tensor.matmul(
                    ps[:, 0:512], lt, rhs[:, 0:512], start=True, stop=True
                )
                nc.tensor.matmul(
                    ps[:, 512:1024], lt, rhs[:, 512:1024], start=True, stop=True
                )
                sg = sigpool.tile([M, N], F32, tag="sg", name="sg")
                nc.scalar.activation(
                    sg[:],
                    ps[:],
                    AF.Sigmoid,
                    bias=vt,
                    accum_out=z_t[br][:, b : b + 1],
                )

        # ---- MLP head: h = sigmoid(W_h @ z); y = C_w @ h + C_b ----
        h_t = []
        for hh in range(2):
            hp = mlppool.tile([128, B_LOC], F32, tag="mlp", name="hp")
            nc.tensor.matmul(
                hp[:],
                pack[:, PK_WHT0 + hh * 128 : PK_WHT0 + (hh + 1) * 128],
                z_t[0][:],
                start=True,
                stop=False,
            )
            nc.tensor.matmul(
                hp[:],
                pack[:, PK_WHT1 + hh * 128 : PK_WHT1 + (hh + 1) * 128],
                z_t[1][:],
                start=False,
                stop=True,
            )
            hs = spool.tile([128, B_LOC], F32, tag=f"hs{hh}", name="hs")
            nc.scalar.activation(hs[:], hp[:], AF.Sigmoid)
            h_t.append(hs)

        yp = mlppool.tile([B_LOC, O], F32, tag="mlp", name="yp")
        nc.tensor.matmul(
            yp[:], h_t[0][:], pack[:, PK_CW0 : PK_CW0 + O], start=True, stop=False
        )
        nc.tensor.matmul(
            yp[:], h_t[1][:], pack[:, PK_CW1 : PK_CW1 + O], start=False, stop=False
        )
        nc.tensor.matmul(yp[:], ones_t[:], cb_t[:], start=False, stop=True)
        y_sb = spool.tile([B_LOC, O], F32, tag="ysb", name="ysb")
        nc.vector.tensor_copy(y_sb[:], yp[:])
        nc.sync.dma_start(io["y"], y_sb[:])


_CACHED = None


def _build():
    global _CACHED
    if _CACHED is not None:
        return _CACHED
    nc = bacc.Bacc(
        "TRN2", target_bir_lowering=False, debug=False, num_devices=NCORES
    )
    io = {}
    io["x0"] = nc.dram_tensor("x0", [128, N], F32, kind="ExternalInput").ap()
    io["x1"] = nc.dram_tensor("x1", [128, N], F32, kind="ExternalInput").ap()
    io["pack"] = nc.dram_tensor(
        "pack", [128, PK_COLS], F32, kind="ExternalInput"
    ).ap()
    io["lhs"] = nc.dram_tensor("lhs", [8, 2 * M], BF16, kind="ExternalInput").ap()
    io["cb"] = nc.dram_tensor("cb", [1, O], F32, kind="ExternalInput").ap()
    io["y"] = nc.dram_tensor("y", [B_LOC, O], F32, kind="ExternalOutput").ap()

    with tile.TileContext(nc) as tc:
        _emit(tc, io)
    nc.compile()
    _CACHED = nc
    return nc


def _prep_params(inputs):
    import ml_dtypes

    f = np.float32
    bf = ml_dtypes.bfloat16

    def branch_lhs(P, U, A):
        W_g = (U @ A).reshape(M, L).astype(np.float64)
        w = (W_g @ P.astype(np.float64))[:, 0]
        his, los = [], []
        for part in (np.maximum(w, 0.0), np.minimum(w, 0.0)):
            hi = part.astype(f).astype(bf)
            lo = (part.astype(f) - hi.astype(f)).astype(bf)
            his.append(hi)
            los.append(lo)
        # rows: wph wph wnh wnh wpl wpl wnl wnl (pair with R's
        # xmh xml xnh xnl xmh xml xnh xnl)
        rows = [his[0], his[0], his[1], his[1], los[0], los[0], los[1], los[1]]
        return np.stack(rows).astype(bf)

    pack = np.zeros((128, PK_COLS), dtype=f)
    pack[:, PK_WHT0 : PK_WHT0 + 256] = inputs["W_h"].T[0:128, :]
    pack[:, PK_WHT1 : PK_WHT1 + 256] = inputs["W_h"].T[128:256, :]
    pack[:, PK_CW0 : PK_CW0 + O] = inputs["C_w"].T[0:128, :]
    pack[:, PK_CW1 : PK_CW1 + O] = inputs["C_w"].T[128:256, :]
    pack[:, PK_V0] = inputs["V0"].astype(f)
    pack[:, PK_V1] = inputs["V1"].astype(f)

    lhs = np.concatenate(
        [
            branch_lhs(inputs["P0"], inputs["U0"], inputs["A0"]),
            branch_lhs(inputs["P1"], inputs["U1"], inputs["A1"]),
        ],
        axis=1,
    )

    return {
        "pack": pack,
        "lhs": np.ascontiguousarray(lhs),
        "cb": np.ascontiguousarray(inputs["C_b"].reshape(1, O), dtype=f),
    }


def run(inputs, trace=False, **kw):
    nc = _build()
    params = _prep_params(inputs)
    x0 = np.ascontiguousarray(inputs["x0"], dtype=np.float32)
    x1 = np.ascontiguousarray(inputs["x1"], dtype=np.float32)
    in_maps = []
    for c in range(NCORES):
        m = dict(params)
        m["x0"] = x0[c * B_LOC : (c + 1) * B_LOC].reshape(128, N)
        m["x1"] = x1[c * B_LOC : (c + 1) * B_LOC].reshape(128, N)
        in_maps.append(m)
    res = run_bass_kernel_spmd(nc, in_maps, list(range(NCORES)), trace=trace, **kw)
    y = np.concatenate([res.results[c]["y"] for c in range(NCORES)], axis=0)
    return y, res


def kernel(**inputs):
    y, _ = run(inputs, trace=False)
    return y


# revision 19
# speedup vs baseline: 1.2796x; 1.1775x over previous
"""Trainium2 Bass kernel for nn_EquiPINE (pooling).

Math (per branch):
    W_g = (U @ A).reshape(M, L); w = W_g @ P  -> [M]
    g = sigmoid(x[...,None] * w + V)          -> [B, N, D, M]
    out = sum_n max_d g                       -> [B, M]

Key restructuring: sigmoid is monotonic, so
    max_d sigmoid(x*w + V) = sigmoid(max_d(x*w) + V)
and max_d(x[b,n,d]*w[m]) = w_pos[m]*xmax[b,n] + w_neg[m]*xmin[b,n]
(with w_pos = max(w,0), w_neg = min(w,0)).  With xmax/xmin and the w
vectors each split into bf16 hi+lo pairs (bf16x2 ~= fp32 precision at
the PE's 1 cycle/row bf16 rate), the whole [B,N,D,M] intermediate
collapses into one K=8 matmul -> PSUM [128, N] per (batch, branch),
followed by one ACT sigmoid (+per-partition bias V, accumulate-over-
free) producing the pooled z column directly.

Sharding: data-parallel over batch; 8 batches per core on 8 cores.
Params are tiny and replicated; host precomputes w hi/lo rows, W_h^T,
C_w^T and packs all f32 params into one DMA (HWDGE issue costs ~625ns
of engine time per dma_start, so DMA count dominates the prep phase).
"""

import numpy as np

import concourse.bass as bass
import concourse.bacc as bacc
import concourse.tile as tile
from concourse import mybir
from concourse.bass_utils import run_bass_kernel_spmd

NCORES = 8
B = 64
B_LOC = B // NCORES  # 8 batches per core
N = 1024
D = 16
M = 128
L = 32
H = 256
O = 128

F32 = mybir.dt.float32
BF16 = mybir.dt.bfloat16
AF = mybir.ActivationFunctionType
ALU = mybir.AluOpType
AX = mybir.AxisListType

# packed f32 param column layout: wht0 wht1 cw0 cw1 v0 v1
PK_WHT0 = 0
PK_WHT1 = 256
PK_CW0 = 512
PK_CW1 = 640
PK_V0 = 768
PK_V1 = 769
PK_COLS = 770


def _emit(tc, io):
    nc = tc.nc
    with (
        tc.tile_pool(name="const", bufs=1) as cpool,
        tc.tile_pool(name="xp", bufs=1) as xpool,
        tc.tile_pool(name="stat", bufs=1) as spool,
        tc.tile_pool(name="sig", bufs=2) as sigpool,
        tc.tile_pool(name="ps", bufs=3, space="PSUM") as pspool,
        tc.tile_pool(name="psmlp", bufs=2, space="PSUM") as mlppool,
    ):
        # ---- x loads first, split into halves for parallel transfer;
        # x0 on sync, x1 on scalar, params on the gpsimd SWDGE ----
        xt = []
        x_eng = [nc.sync, nc.scalar]
        for xi in range(2):
            t = xpool.tile([128, N], F32, tag=f"x{xi}", name=f"xt{xi}")
            x_eng[xi].dma_start(t[:, 0:512], io[f"x{xi}"][:, 0:512])
            x_eng[xi].dma_start(t[:, 512:1024], io[f"x{xi}"][:, 512:1024])
            xt.append(t)
        pack = cpool.tile([128, PK_COLS], F32, tag="pack")
        nc.gpsimd.dma_start(pack[:], io["pack"])
        lhs = cpool.tile([4, 2 * M], BF16, tag="lhs")
        nc.gpsimd.dma_start(lhs[:], io["lhs"])
        cb_t = cpool.tile([1, O], F32, tag="cb")
        nc.gpsimd.dma_start(cb_t[:], io["cb"])
        ones_t = cpool.tile([1, B_LOC], F32, tag="ones")
        nc.gpsimd.memset(ones_t[:], 1.0)

        # z columns per branch: [M, B_LOC]
        z_t = [
            cpool.tile([M, B_LOC], F32, tag=f"z{br}", name=f"z{br}")
            for br in range(2)
        ]

        # ---- per x-tensor: d-reduce (max/min), bf16 cast, row shuffle ----
        # x shard viewed as [128, 1024]: partition p = b*16 + n//64,
        # free f = (n%64)*16 + d.  xmax/xmin rounding to bf16 is random
        # across n and averages out in the sigmoid sum (verified 4e-7
        # end-to-end), so no hi/lo split needed for x - only for w.
        r_t = []
        for xi in range(2):
            x3 = xt[xi][:].rearrange("p (c d) -> p c d", d=D)
            xmax = spool.tile([128, 64], F32, tag=f"xmax{xi}", name="xmax")
            xmin = spool.tile([128, 64], F32, tag=f"xmin{xi}", name="xmin")
            nc.vector.tensor_reduce(xmax[:], x3, axis=AX.X, op=ALU.max)
            nc.vector.tensor_reduce(xmin[:], x3, axis=AX.X, op=ALU.min)
            # comb columns: bf16(xmax) | bf16(xmin)
            comb = spool.tile([128, 128], BF16, tag=f"comb{xi}", name="comb")
            nc.vector.tensor_copy(comb[:, 0:64], xmax[:])
            nc.vector.tensor_copy(comb[:, 64:128], xmin[:])
            # R rows = xm xm xn xn; free = b*1024 + c*64 + j (n = c*64+j)
            rt = spool.tile([4, B_LOC * N], BF16, tag=f"r{xi}", name="rt")
            for row, src in enumerate([0, 0, 1, 1]):
                x_eng[row % 2].dma_start(
                    rt[row : row + 1, :].rearrange(
                        "p (b c j) -> p b c j", c=16, j=64
                    ),
                    comb[:, src * 64 : (src + 1) * 64],
                )
            r_t.append(rt)

        # ---- branch core: K=4 matmul + fused sigmoid/bias/accum ----
        # lhs rows: wph wpl wnh wnl pair with rt rows: xm xm xn xn
        for br in range(2):
            lt = lhs[:, br * M : (br + 1) * M]
            vt = pack[:, PK_V0 + br : PK_V0 + br + 1]
            for b in range(B_LOC):
                ps = pspool.tile([M, N], F32, tag="s", name="ps")
                rhs = r_t[br][:, b * N : (b + 1) * N]
                nc.tensor.matmul(
                    ps[:, 0:512], lt, rhs[:, 0:512], start=True, stop=True
                )
                nc.tensor.matmul(
                    ps[:, 512:1024], lt, rhs[:, 512:1024], start=True, stop=True
                )
                sg = sigpool.tile([M, N], F32, tag="sg", name="sg")
                nc.scalar.activation(
                    sg[:],
                    ps[:],
                    AF.Sigmoid,
                    bias=vt,
                    accum_out=z_t[br][:, b : b + 1],
                )

        # ---- MLP head: h = sigmoid(W_h @ z); y = C_w @ h + C_b ----
        h_t = []
        for hh in range(2):
            hp = mlppool.tile([128, B_LOC], F32, tag="mlp", name="hp")
            nc.tensor.matmul(
                hp[:],
                pack[:, PK_WHT0 + hh * 128 : PK_WHT0 + (hh + 1) * 128],
                z_t[0][:],
                start=True,
                stop=False,
            )
            nc.tensor.matmul(
                hp[:],
                pack[:, PK_WHT1 + hh * 128 : PK_WHT1 + (hh + 1) * 128],
                z_t[1][:],
                start=False,
                stop=True,
            )
            hs = spool.tile([128, B_LOC], F32, tag=f"hs{hh}", name="hs")
            nc.scalar.activation(hs[:], hp[:], AF.Sigmoid)
            h_t.append(hs)

        yp = mlppool.tile([B_LOC, O], F32, tag="mlp", name="yp")
        nc.tensor.matmul(
            yp[:], h_t[0][:], pack[:, PK_CW0 : PK_CW0 + O], start=True, stop=False
        )
        nc.tensor.matmul(
            yp[:], h_t[1][:], pack[:, PK_CW1 : PK_CW1 + O], start=False, stop=False
        )
        nc.tensor.matmul(yp[:], ones_t[:], cb_t[:], start=False, stop=True)
        y_sb = spool.tile([B_LOC, O], F32, tag="ysb", name="ysb")
        nc.vector.tensor_copy(y_sb[:], yp[:])
        nc.sync.dma_start(io["y"], y_sb[:])


_CACHED = None


def _build():
    global _CACHED
    if _CACHED is not None:
        return _CACHED
    nc = bacc.Bacc(
        "TRN2", target_bir_lowering=False, debug=False, num_devices=NCORES
    )
    io = {}
    io["x0"] = nc.dram_tensor("x0", [128, N], F32, kind="ExternalInput").ap()
    io["x1"] = nc.dram_tensor("x1", [128, N], F32, kind="ExternalInput").ap()
    io["pack"] = nc.dram_tensor(
        "pack", [128, PK_COLS], F32, kind="ExternalInput"
    ).ap()
    io["lhs"] = nc.dram_tensor("lhs", [4, 2 * M], BF16, kind="ExternalInput").ap()
    io["cb"] = nc.dram_tensor("cb", [1, O], F32, kind="ExternalInput").ap()
    io["y"] = nc.dram_tensor("y", [B_LOC, O], F32, kind="ExternalOutput").ap()

    with tile.TileContext(nc) as tc:
        _emit(tc, io)
    nc.compile()
    _CACHED = nc
    return nc


def _prep_params(inputs):
    import ml_dtypes

    f = np.float32
    bf = ml_dtypes.bfloat16

    def branch_lhs(P, U, A):
        W_g = (U @ A).reshape(M, L).astype(np.float64)
        w = (W_g @ P.astype(np.float64))[:, 0]
        rows = []
        for part in (np.maximum(w, 0.0), np.minimum(w, 0.0)):
            hi = part.astype(f).astype(bf)
            lo = (part.astype(f) - hi.astype(f)).astype(bf)
            rows += [hi, lo]
        # rows: wph wpl wnh wnl (pair with R's xm xm xn xn)
        return np.stack(rows).astype(bf)

    pack = np.zeros((128, PK_COLS), dtype=f)
    pack[:, PK_WHT0 : PK_WHT0 + 256] = inputs["W_h"].T[0:128, :]
    pack[:, PK_WHT1 : PK_WHT1 + 256] = inputs["W_h"].T[128:256, :]
    pack[:, PK_CW0 : PK_CW0 + O] = inputs["C_w"].T[0:128, :]
    pack[:, PK_CW1 : PK_CW1 + O] = inputs["C_w"].T[128:256, :]
    pack[:, PK_V0] = inputs["V0"].astype(f)
    pack[:, PK_V1] = inputs["V1"].astype(f)

    lhs = np.concatenate(
        [
            branch_lhs(inputs["P0"], inputs["U0"], inputs["A0"]),
            branch_lhs(inputs["P1"], inputs["U1"], inputs["A1"]),
        ],
        axis=1,
    )

    return {
        "pack": pack,
        "lhs": np.ascontiguousarray(lhs),
        "cb": np.ascontiguousarray(inputs["C_b"].reshape(1, O), dtype=f),
    }


def run(inputs, trace=False, **kw):
    nc = _build()
    params = _prep_params(inputs)
    x0 = np.ascontiguousarray(inputs["x0"], dtype=np.float32)
    x1 = np.ascontiguousarray(inputs["x1"], dtype=np.float32)
    in_maps = []
    for c in range(NCORES):
        m = dict(params)
        m["x0"] = x0[c * B_LOC : (c + 1) * B_LOC].reshape(128, N)
        m["x1"] = x1[c * B_LOC : (c + 1) * B_LOC].reshape(128, N)
        in_maps.append(m)
    res = run_bass_kernel_spmd(nc, in_maps, list(range(NCORES)), trace=trace, **kw)
    y = np.concatenate([res.results[c]["y"] for c in range(NCORES)], axis=0)
    return y, res


def kernel(**inputs):
    y, _ = run(inputs, trace=False)
    return y


# revision 23
# speedup vs baseline: 1.2846x; 1.0039x over previous
"""Trainium2 Bass kernel for nn_EquiPINE (pooling).

Math (per branch):
    W_g = (U @ A).reshape(M, L); w = W_g @ P  -> [M]
    g = sigmoid(x[...,None] * w + V)          -> [B, N, D, M]
    out = sum_n max_d g                       -> [B, M]

Key restructuring: sigmoid is monotonic, so
    max_d sigmoid(x*w + V) = sigmoid(max_d(x*w) + V)
and max_d(x[b,n,d]*w[m]) = w_pos[m]*xmax[b,n] + w_neg[m]*xmin[b,n]
(with w_pos = max(w,0), w_neg = min(w,0)).  w is split into bf16 hi+lo
(its rounding error would be systematic across the n-sum); xmax/xmin
go to plain bf16 (their rounding is random across n and averages out -
verified 4e-7 end-to-end).  The whole [B,N,D,M] intermediate collapses
into one K=4 bf16 matmul -> PSUM [128, N] per (batch, branch), then one
ACT sigmoid (+per-partition bias V, accumulate-over-free) producing the
pooled z column directly.

Sharding: data-parallel over batch; 8 batches per core on 8 cores.
Params are tiny and replicated; host precomputes w hi/lo rows, W_h^T,
C_w^T and packs all f32 params into one DMA.

DMA budget notes: every hwdge dma_start costs ~625ns of issuing-engine
time and all of an engine's transfers share one hardware queue
(~90-200 GB/s), so transfers are split across the sync/scalar/gpsimd
queues and the scalar engine is kept DMA-free once the sigmoid phase
starts.
"""

import numpy as np

import concourse.bass as bass
import concourse.bacc as bacc
import concourse.tile as tile
from concourse import mybir
from concourse.bass_utils import run_bass_kernel_spmd

NCORES = 8
B = 64
B_LOC = B // NCORES  # 8 batches per core
N = 1024
D = 16
M = 128
L = 32
H = 256
O = 128

F32 = mybir.dt.float32
BF16 = mybir.dt.bfloat16
AF = mybir.ActivationFunctionType
ALU = mybir.AluOpType
AX = mybir.AxisListType

# packed f32 param column layout: wht0 wht1 cw0 cw1 v0 v1
PK_WHT0 = 0
PK_WHT1 = 256
PK_CW0 = 512
PK_CW1 = 640
PK_V0 = 768
PK_V1 = 769
PK_COLS = 770


def _emit(tc, io):
    nc = tc.nc
    with (
        tc.tile_pool(name="const", bufs=1) as cpool,
        tc.tile_pool(name="xp", bufs=1) as xpool,
        tc.tile_pool(name="stat", bufs=1) as spool,
        tc.tile_pool(name="sig", bufs=2) as sigpool,
        tc.tile_pool(name="ps", bufs=3, space="PSUM") as pspool,
        tc.tile_pool(name="psmlp", bufs=2, space="PSUM") as mlppool,
    ):
        # ---- loads: x0 halves split across the sync+scalar queues for
        # parallel transfer; x1 on the gpsimd SWDGE queue; params early
        # on scalar (all scalar DMA gen finishes before sigmoids start).
        xt0 = xpool.tile([128, N], F32, tag="x0", name="xt0")
        nc.sync.dma_start(xt0[:, 0:512], io["x0"][:, 0:512])
        lhs = cpool.tile([4, 2 * M], BF16, tag="lhs")
        nc.scalar.dma_start(lhs[:], io["lhs"])
        nc.scalar.dma_start(xt0[:, 512:1024], io["x0"][:, 512:1024])
        xt1 = xpool.tile([128, N], F32, tag="x1", name="xt1")
        nc.gpsimd.dma_start(xt1[:], io["x1"])
        cb_t = cpool.tile([1, O], F32, tag="cb")
        nc.scalar.dma_start(cb_t[:], io["cb"])
        pack = cpool.tile([128, PK_COLS], F32, tag="pack")
        nc.scalar.dma_start(pack[:], io["pack"])
        ones_t = cpool.tile([1, B_LOC], F32, tag="ones")
        nc.vector.memset(ones_t[:], 1.0)

        # z columns per branch: [M, B_LOC]
        z_t = [
            cpool.tile([M, B_LOC], F32, tag=f"z{br}", name=f"z{br}")
            for br in range(2)
        ]

        # ---- per x-tensor: d-reduce (max/min), bf16 cast, row shuffle ----
        # x shard viewed as [128, 1024]: partition p = b*16 + n//64,
        # free f = (n%64)*16 + d.
        xt = [xt0, xt1]
        # x0 rows: 2 on sync + 2 on scalar (prep phase, both idle);
        # x1 rows: all on sync (scalar is running sigmoids by then).
        row_eng = [[nc.sync, nc.scalar, nc.sync, nc.scalar],
                   [nc.sync, nc.sync, nc.sync, nc.sync]]
        r_t = []
        for xi in range(2):
            x3 = xt[xi][:].rearrange("p (c d) -> p c d", d=D)
            xmax = spool.tile([128, 64], F32, tag=f"xmax{xi}", name="xmax")
            xmin = spool.tile([128, 64], F32, tag=f"xmin{xi}", name="xmin")
            nc.vector.tensor_reduce(xmax[:], x3, axis=AX.X, op=ALU.max)
            nc.vector.tensor_reduce(xmin[:], x3, axis=AX.X, op=ALU.min)
            # comb columns: bf16(xmax) | bf16(xmin)
            comb = spool.tile([128, 128], BF16, tag=f"comb{xi}", name="comb")
            nc.vector.tensor_copy(comb[:, 0:64], xmax[:])
            nc.vector.tensor_copy(comb[:, 64:128], xmin[:])
            # R rows = xm xm xn xn; free = b*1024 + c*64 + j (n = c*64+j)
            rt = spool.tile([4, B_LOC * N], BF16, tag=f"r{xi}", name="rt")
            for row, src in enumerate([0, 0, 1, 1]):
                row_eng[xi][row].dma_start(
                    rt[row : row + 1, :].rearrange(
                        "p (b c j) -> p b c j", c=16, j=64
                    ),
                    comb[:, src * 64 : (src + 1) * 64],
                )
            r_t.append(rt)

        # ---- branch core: K=4 matmul + fused sigmoid/bias/accum ----
        # lhs rows: wph wpl wnh wnl pair with rt rows: xm xm xn xn
        def unit(br, b):
            lt = lhs[:, br * M : (br + 1) * M]
            vt = pack[:, PK_V0 + br : PK_V0 + br + 1]
            ps = pspool.tile([M, N], F32, tag="s", name="ps")
            rhs = r_t[br][:, b * N : (b + 1) * N]
            nc.tensor.matmul(ps[:, 0:512], lt, rhs[:, 0:512], start=True, stop=True)
            nc.tensor.matmul(
                ps[:, 512:1024], lt, rhs[:, 512:1024], start=True, stop=True
            )
            sg = sigpool.tile([M, N], F32, tag="sg", name="sg")
            nc.scalar.activation(
                sg[:],
                ps[:],
                AF.Sigmoid,
                bias=vt,
                accum_out=z_t[br][:, b : b + 1],
            )

        # ---- MLP head (per batch-half so half 0 hides under sigmoids):
        # h = sigmoid(W_h @ z); y = C_w @ h + C_b ----
        y_half = [
            spool.tile([4, O], F32, tag=f"ysb{h}", name=f"ysb{h}")
            for h in range(2)
        ]

        def mlp_half(half):
            bs = slice(half * 4, half * 4 + 4)
            h_t = []
            for hh in range(2):
                hp = mlppool.tile([128, 4], F32, tag="mlp", name="hp")
                nc.tensor.matmul(
                    hp[:],
                    pack[:, PK_WHT0 + hh * 128 : PK_WHT0 + (hh + 1) * 128],
                    z_t[0][:, bs],
                    start=True,
                    stop=False,
                )
                nc.tensor.matmul(
                    hp[:],
                    pack[:, PK_WHT1 + hh * 128 : PK_WHT1 + (hh + 1) * 128],
                    z_t[1][:, bs],
                    start=False,
                    stop=True,
                )
                hs = spool.tile([128, 4], F32, tag=f"hs{hh}_{half}", name="hs")
                nc.scalar.activation(hs[:], hp[:], AF.Sigmoid)
                h_t.append(hs)
            yp = mlppool.tile([4, O], F32, tag="mlp", name="yp")
            nc.tensor.matmul(
                yp[:], h_t[0][:], pack[:, PK_CW0 : PK_CW0 + O], start=True, stop=False
            )
            nc.tensor.matmul(
                yp[:], h_t[1][:], pack[:, PK_CW1 : PK_CW1 + O], start=False, stop=False
            )
            nc.tensor.matmul(
                yp[:], ones_t[:, 0:4], cb_t[:], start=False, stop=True
            )
            nc.vector.tensor_copy(y_half[half][:], yp[:])

        for b in range(B_LOC):
            unit(0, b)
        for b in range(B_LOC):
            unit(1, b)
            if b == 3:
                mlp_half(0)
        mlp_half(1)
        nc.sync.dma_start(io["y"][0:4, :], y_half[0][:])
        nc.sync.dma_start(io["y"][4:8, :], y_half[1][:])


_CACHED = None


def _build():
    global _CACHED
    if _CACHED is not None:
        return _CACHED
    nc = bacc.Bacc(
        "TRN2", target_bir_lowering=False, debug=False, num_devices=NCORES
    )
    io = {}
    io["x0"] = nc.dram_tensor("x0", [128, N], F32, kind="ExternalInput").ap()
    io["x1"] = nc.dram_tensor("x1", [128, N], F32, kind="ExternalInput").ap()
    io["pack"] = nc.dram_tensor(
        "pack", [128, PK_COLS], F32, kind="ExternalInput"
    ).ap()
    io["lhs"] = nc.dram_tensor("lhs", [4, 2 * M], BF16, kind="ExternalInput").ap()
    io["cb"] = nc.dram_tensor("cb", [1, O], F32, kind="ExternalInput").ap()
    io["y"] = nc.dram_tensor("y", [B_LOC, O], F32, kind="ExternalOutput").ap()

    with tile.TileContext(nc) as tc:
        _emit(tc, io)
    nc.compile()
    _CACHED = nc
    return nc


def _prep_params(inputs):
    import ml_dtypes

    f = np.float32
    bf = ml_dtypes.bfloat16

    def branch_lhs(P, U, A):
        W_g = (U @ A).reshape(M, L).astype(np.float64)
        w = (W_g @ P.astype(np.float64))[:, 0]
        rows = []
        for part in (np.maximum(w, 0.0), np.minimum(w, 0.0)):
            hi = part.astype(f).astype(bf)
            lo = (part.astype(f) - hi.astype(f)).astype(bf)
            rows += [hi, lo]
        # rows: wph wpl wnh wnl (pair with R's xm xm xn xn)
        return np.stack(rows).astype(bf)

    pack = np.zeros((128, PK_COLS), dtype=f)
    pack[:, PK_WHT0 : PK_WHT0 + 256] = inputs["W_h"].T[0:128, :]
    pack[:, PK_WHT1 : PK_WHT1 + 256] = inputs["W_h"].T[128:256, :]
    pack[:, PK_CW0 : PK_CW0 + O] = inputs["C_w"].T[0:128, :]
    pack[:, PK_CW1 : PK_CW1 + O] = inputs["C_w"].T[128:256, :]
    pack[:, PK_V0] = inputs["V0"].astype(f)
    pack[:, PK_V1] = inputs["V1"].astype(f)

    lhs = np.concatenate(
        [
            branch_lhs(inputs["P0"], inputs["U0"], inputs["A0"]),
            branch_lhs(inputs["P1"], inputs["U1"], inputs["A1"]),
        ],
        axis=1,
    )

    return {
        "pack": pack,
        "lhs": np.ascontiguousarray(lhs),
        "cb": np.ascontiguousarray(inputs["C_b"].reshape(1, O), dtype=f),
    }


def run(inputs, trace=False, **kw):
    nc = _build()
    params = _prep_params(inputs)
    x0 = np.ascontiguousarray(inputs["x0"], dtype=np.float32)
    x1 = np.ascontiguousarray(inputs["x1"], dtype=np.float32)
    in_maps = []
    for c in range(NCORES):
        m = dict(params)
        m["x0"] = x0[c * B_LOC : (c + 1) * B_LOC].reshape(128, N)
        m["x1"] = x1[c * B_LOC : (c + 1) * B_LOC].reshape(128, N)
        in_maps.append(m)
    res = run_bass_kernel_spmd(nc, in_maps, list(range(NCORES)), trace=trace, **kw)
    y = np.concatenate([res.results[c]["y"] for c in range(NCORES)], axis=0)
    return y, res


def kernel(**inputs):
    y, _ = run(inputs, trace=False)
    return y


# revision 26
# speedup vs baseline: 1.3147x; 1.0235x over previous
"""Trainium2 Bass kernel for nn_EquiPINE (pooling).

Math (per branch):
    W_g = (U @ A).reshape(M, L); w = W_g @ P  -> [M]
    g = sigmoid(x[...,None] * w + V)          -> [B, N, D, M]
    out = sum_n max_d g                       -> [B, M]

Key restructuring: sigmoid is monotonic, so
    max_d sigmoid(x*w + V) = sigmoid(max_d(x*w) + V)
and max_d(x[b,n,d]*w[m]) = w_pos[m]*xmax[b,n] + w_neg[m]*xmin[b,n]
(with w_pos = max(w,0), w_neg = min(w,0)).  w is split into bf16 hi+lo
(its rounding error would be systematic across the n-sum); xmax/xmin
go to plain bf16 (their rounding is random across n and averages out -
verified 4e-7 end-to-end).  The whole [B,N,D,M] intermediate collapses
into one K=4 bf16 matmul -> PSUM [128, N] per (batch, branch), then one
ACT sigmoid (+per-partition bias V, accumulate-over-free) producing the
pooled z column directly.

Sharding: data-parallel over batch; 8 batches per core on 8 cores.
Params are tiny and replicated; host precomputes w hi/lo rows, W_h^T,
C_w^T and packs all f32 params into one DMA.

DMA budget notes: every hwdge dma_start costs ~625ns of issuing-engine
time and all of an engine's transfers share one hardware queue
(~90-200 GB/s), so transfers are split across the sync/scalar/gpsimd
queues and the scalar engine is kept DMA-free once the sigmoid phase
starts.
"""

import numpy as np

import concourse.bass as bass
import concourse.bacc as bacc
import concourse.tile as tile
from concourse import mybir
from concourse.bass_utils import run_bass_kernel_spmd

NCORES = 8
B = 64
B_LOC = B // NCORES  # 8 batches per core
N = 1024
D = 16
M = 128
L = 32
H = 256
O = 128

F32 = mybir.dt.float32
BF16 = mybir.dt.bfloat16
AF = mybir.ActivationFunctionType
ALU = mybir.AluOpType
AX = mybir.AxisListType

# packed f32 param column layout: wht0 wht1 cw0 cw1 v0 v1
PK_WHT0 = 0
PK_WHT1 = 256
PK_CW0 = 512
PK_CW1 = 640
PK_V0 = 768
PK_V1 = 769
PK_COLS = 770


def _emit(tc, io):
    nc = tc.nc
    with (
        tc.tile_pool(name="const", bufs=1) as cpool,
        tc.tile_pool(name="xp", bufs=1) as xpool,
        tc.tile_pool(name="stat", bufs=1) as spool,
        tc.tile_pool(name="sig", bufs=2) as sigpool,
        tc.tile_pool(name="ps", bufs=3, space="PSUM") as pspool,
        tc.tile_pool(name="psmlp", bufs=2, space="PSUM") as mlppool,
    ):
        # ---- loads: x0 halves split across the sync+scalar queues for
        # parallel transfer; x1 on the gpsimd SWDGE queue; params early
        # on scalar (all scalar DMA gen finishes before sigmoids start).
        # partition-split halves stay contiguous in HBM (column splits
        # produce strided 2KB descriptors and halve the queue bandwidth)
        xt0 = xpool.tile([128, N], F32, tag="x0", name="xt0")
        xt1 = xpool.tile([128, N], F32, tag="x1", name="xt1")
        nc.sync.dma_start(xt0[0:64, :], io["x0"][0:64, :])
        nc.gpsimd.dma_start(xt0[64:128, :], io["x0"][64:128, :])
        nc.sync.dma_start(xt1[0:64, :], io["x1"][0:64, :])
        nc.gpsimd.dma_start(xt1[64:128, :], io["x1"][64:128, :])
        cb_t = cpool.tile([1, O], F32, tag="cb")
        nc.scalar.dma_start(cb_t[:], io["cb"])
        lhs = cpool.tile([4, 2 * M], BF16, tag="lhs")
        nc.scalar.dma_start(lhs[:], io["lhs"])
        pack = cpool.tile([128, PK_COLS], F32, tag="pack")
        nc.scalar.dma_start(pack[:], io["pack"])
        ones_t = cpool.tile([1, B_LOC], F32, tag="ones")
        nc.vector.memset(ones_t[:], 1.0)

        # z columns per branch: [M, B_LOC]
        z_t = [
            cpool.tile([M, B_LOC], F32, tag=f"z{br}", name=f"z{br}")
            for br in range(2)
        ]

        # ---- per x-tensor: d-reduce (max/min), bf16 cast, row shuffle ----
        # x shard viewed as [128, 1024]: partition p = b*16 + n//64,
        # free f = (n%64)*16 + d.
        xt = [xt0, xt1]
        # x0 rows: 2 on sync + 2 on scalar (prep phase, both idle);
        # x1 rows: all on sync (scalar is running sigmoids by then).
        row_eng = [[nc.sync, nc.scalar, nc.sync, nc.scalar],
                   [nc.sync, nc.sync, nc.sync, nc.sync]]
        from concourse.bass import _add_dep_helper

        r_t = []
        last_cast = None
        for xi in range(2):
            x3 = xt[xi][:].rearrange("p (c d) -> p c d", d=D)
            xmax = spool.tile([128, 64], F32, tag=f"xmax{xi}", name="xmax")
            xmin = spool.tile([128, 64], F32, tag=f"xmin{xi}", name="xmin")
            red0 = nc.vector.tensor_reduce(xmax[:], x3, axis=AX.X, op=ALU.max)
            if last_cast is not None:
                # keep x1's reduces behind x0's casts in the static DVE
                # stream (the scheduler otherwise interleaves them and
                # head-of-line-blocks x0's row shuffle for ~2us)
                _add_dep_helper(
                    red0.ins, last_cast.ins, sync=False, reason="x0 casts first"
                )
            nc.vector.tensor_reduce(xmin[:], x3, axis=AX.X, op=ALU.min)
            # comb columns: bf16(xmax) | bf16(xmin)
            comb = spool.tile([128, 128], BF16, tag=f"comb{xi}", name="comb")
            nc.vector.tensor_copy(comb[:, 0:64], xmax[:])
            last_cast = nc.vector.tensor_copy(comb[:, 64:128], xmin[:])
            # R rows = xm xm xn xn; free = b*1024 + c*64 + j (n = c*64+j)
            rt = spool.tile([4, B_LOC * N], BF16, tag=f"r{xi}", name="rt")
            for row, src in enumerate([0, 0, 1, 1]):
                row_eng[xi][row].dma_start(
                    rt[row : row + 1, :].rearrange(
                        "p (b c j) -> p b c j", c=16, j=64
                    ),
                    comb[:, src * 64 : (src + 1) * 64],
                )
            r_t.append(rt)

        # ---- branch core: K=4 matmul + fused sigmoid/bias/accum ----
        # lhs rows: wph wpl wnh wnl pair with rt rows: xm xm xn xn
        def unit(br, b):
            lt = lhs[:, br * M : (br + 1) * M]
            vt = pack[:, PK_V0 + br : PK_V0 + br + 1]
            ps = pspool.tile([M, N], F32, tag="s", name="ps")
            rhs = r_t[br][:, b * N : (b + 1) * N]
            nc.tensor.matmul(ps[:, 0:512], lt, rhs[:, 0:512], start=True, stop=True)
            nc.tensor.matmul(
                ps[:, 512:1024], lt, rhs[:, 512:1024], start=True, stop=True
            )
            sg = sigpool.tile([M, N], F32, tag="sg", name="sg")
            nc.scalar.activation(
                sg[:],
                ps[:],
                AF.Sigmoid,
                bias=vt,
                accum_out=z_t[br][:, b : b + 1],
            )

        # ---- MLP head (per batch-half so half 0 hides under sigmoids):
        # h = sigmoid(W_h @ z); y = C_w @ h + C_b ----
        y_half = [
            spool.tile([4, O], F32, tag=f"ysb{h}", name=f"ysb{h}")
            for h in range(2)
        ]

        def mlp_half(half):
            bs = slice(half * 4, half * 4 + 4)
            h_t = []
            for hh in range(2):
                hp = mlppool.tile([128, 4], F32, tag="mlp", name="hp")
                nc.tensor.matmul(
                    hp[:],
                    pack[:, PK_WHT0 + hh * 128 : PK_WHT0 + (hh + 1) * 128],
                    z_t[0][:, bs],
                    start=True,
                    stop=False,
                )
                nc.tensor.matmul(
                    hp[:],
                    pack[:, PK_WHT1 + hh * 128 : PK_WHT1 + (hh + 1) * 128],
                    z_t[1][:, bs],
                    start=False,
                    stop=True,
                )
                hs = spool.tile([128, 4], F32, tag=f"hs{hh}_{half}", name="hs")
                nc.scalar.activation(hs[:], hp[:], AF.Sigmoid)
                h_t.append(hs)
            yp = mlppool.tile([4, O], F32, tag="mlp", name="yp")
            nc.tensor.matmul(
                yp[:], h_t[0][:], pack[:, PK_CW0 : PK_CW0 + O], start=True, stop=False
            )
            nc.tensor.matmul(
                yp[:], h_t[1][:], pack[:, PK_CW1 : PK_CW1 + O], start=False, stop=False
            )
            nc.tensor.matmul(
                yp[:], ones_t[:, 0:4], cb_t[:], start=False, stop=True
            )
            nc.vector.tensor_copy(y_half[half][:], yp[:])

        for b in range(B_LOC):
            unit(0, b)
        for b in range(B_LOC):
            unit(1, b)
            if b == 3:
                mlp_half(0)
        mlp_half(1)
        nc.sync.dma_start(io["y"][0:4, :], y_half[0][:])
        nc.scalar.dma_start(io["y"][4:8, :], y_half[1][:])


_CACHED = None


def _build():
    global _CACHED
    if _CACHED is not None:
        return _CACHED
    nc = bacc.Bacc(
        "TRN2", target_bir_lowering=False, debug=False, num_devices=NCORES
    )
    io = {}
    io["x0"] = nc.dram_tensor("x0", [128, N], F32, kind="ExternalInput").ap()
    io["x1"] = nc.dram_tensor("x1", [128, N], F32, kind="ExternalInput").ap()
    io["pack"] = nc.dram_tensor(
        "pack", [128, PK_COLS], F32, kind="ExternalInput"
    ).ap()
    io["lhs"] = nc.dram_tensor("lhs", [4, 2 * M], BF16, kind="ExternalInput").ap()
    io["cb"] = nc.dram_tensor("cb", [1, O], F32, kind="ExternalInput").ap()
    io["y"] = nc.dram_tensor("y", [B_LOC, O], F32, kind="ExternalOutput").ap()

    with tile.TileContext(nc) as tc:
        _emit(tc, io)
    nc.compile()
    _CACHED = nc
    return nc


def _prep_params(inputs):
    import ml_dtypes

    f = np.float32
    bf = ml_dtypes.bfloat16

    def branch_lhs(P, U, A):
        W_g = (U @ A).reshape(M, L).astype(np.float64)
        w = (W_g @ P.astype(np.float64))[:, 0]
        rows = []
        for part in (np.maximum(w, 0.0), np.minimum(w, 0.0)):
            hi = part.astype(f).astype(bf)
            lo = (part.astype(f) - hi.astype(f)).astype(bf)
            rows += [hi, lo]
        # rows: wph wpl wnh wnl (pair with R's xm xm xn xn)
        return np.stack(rows).astype(bf)

    pack = np.zeros((128, PK_COLS), dtype=f)
    pack[:, PK_WHT0 : PK_WHT0 + 256] = inputs["W_h"].T[0:128, :]
    pack[:, PK_WHT1 : PK_WHT1 + 256] = inputs["W_h"].T[128:256, :]
    pack[:, PK_CW0 : PK_CW0 + O] = inputs["C_w"].T[0:128, :]
    pack[:, PK_CW1 : PK_CW1 + O] = inputs["C_w"].T[128:256, :]
    pack[:, PK_V0] = inputs["V0"].astype(f)
    pack[:, PK_V1] = inputs["V1"].astype(f)

    lhs = np.concatenate(
        [
            branch_lhs(inputs["P0"], inputs["U0"], inputs["A0"]),
            branch_lhs(inputs["P1"], inputs["U1"], inputs["A1"]),
        ],
        axis=1,
    )

    return {
        "pack": pack,
        "lhs": np.ascontiguousarray(lhs),
        "cb": np.ascontiguousarray(inputs["C_b"].reshape(1, O), dtype=f),
    }


def run(inputs, trace=False, **kw):
    nc = _build()
    params = _prep_params(inputs)
    x0 = np.ascontiguousarray(inputs["x0"], dtype=np.float32)
    x1 = np.ascontiguousarray(inputs["x1"], dtype=np.float32)
    in_maps = []
    for c in range(NCORES):
        m = dict(params)
        m["x0"] = x0[c * B_LOC : (c + 1) * B_LOC].reshape(128, N)
        m["x1"] = x1[c * B_LOC : (c + 1) * B_LOC].reshape(128, N)
        in_maps.append(m)
    res = run_bass_kernel_spmd(nc, in_maps, list(range(NCORES)), trace=trace, **kw)
    y = np.concatenate([res.results[c]["y"] for c in range(NCORES)], axis=0)
    return y, res


def kernel(**inputs):
    y, _ = run(inputs, trace=False)
    return y


# revision 32
# speedup vs baseline: 1.3565x; 1.0318x over previous
"""Trainium2 Bass kernel for nn_EquiPINE (pooling).

Math (per branch):
    W_g = (U @ A).reshape(M, L); w = W_g @ P  -> [M]
    g = sigmoid(x[...,None] * w + V)          -> [B, N, D, M]
    out = sum_n max_d g                       -> [B, M]

Key restructuring: sigmoid is monotonic, so
    max_d sigmoid(x*w + V) = sigmoid(max_d(x*w) + V)
and max_d(x[b,n,d]*w[m]) = w_pos[m]*xmax[b,n] + w_neg[m]*xmin[b,n]
(with w_pos = max(w,0), w_neg = min(w,0)).  w is split into bf16 hi+lo
(its rounding error would be systematic across the n-sum); xmax/xmin
go to plain bf16 (their rounding is random across n and averages out -
verified 4e-7 end-to-end).  The whole [B,N,D,M] intermediate collapses
into one K=4 bf16 matmul -> PSUM [128, N] per (batch, branch), then one
ACT sigmoid (+per-partition bias V, accumulate-over-free) producing the
pooled z column directly.

Sharding: data-parallel over batch; 8 batches per core on 8 cores.
Params are tiny and replicated; host precomputes w hi/lo rows, W_h^T,
C_w^T and packs all f32 params into one DMA.

DMA budget notes: every hwdge dma_start costs ~625ns of issuing-engine
time and all of an engine's transfers share one hardware queue
(~90-200 GB/s), so transfers are split across the sync/scalar/gpsimd
queues and the scalar engine is kept DMA-free once the sigmoid phase
starts.
"""

import numpy as np

import concourse.bass as bass
import concourse.bacc as bacc
import concourse.tile as tile
from concourse import mybir
from concourse.bass_utils import run_bass_kernel_spmd

NCORES = 8
B = 64
B_LOC = B // NCORES  # 8 batches per core
N = 1024
D = 16
M = 128
L = 32
H = 256
O = 128

F32 = mybir.dt.float32
BF16 = mybir.dt.bfloat16
AF = mybir.ActivationFunctionType
ALU = mybir.AluOpType
AX = mybir.AxisListType

# packed f32 param column layout: wht0 wht1 cw0 cw1 v0 v1
PK_WHT0 = 0
PK_WHT1 = 256
PK_CW0 = 512
PK_CW1 = 640
PK_V0 = 768
PK_V1 = 769
PK_COLS = 770


def _emit(tc, io):
    nc = tc.nc
    with (
        tc.tile_pool(name="const", bufs=1) as cpool,
        tc.tile_pool(name="xp", bufs=1) as xpool,
        tc.tile_pool(name="stat", bufs=1) as spool,
        tc.tile_pool(name="sig", bufs=2) as sigpool,
        tc.tile_pool(name="ps", bufs=3, space="PSUM") as pspool,
        tc.tile_pool(name="psmlp", bufs=2, space="PSUM") as mlppool,
    ):
        # ---- loads: x0 halves split across the sync+scalar queues for
        # parallel transfer; x1 on the gpsimd SWDGE queue; params early
        # on scalar (all scalar DMA gen finishes before sigmoids start).
        # x0 arrives as two host-pre-split column blocks (each contiguous
        # in HBM and covering all 128 partitions - partition-split halves
        # would hit the 64-partition SBUF half-bandwidth penalty, column
        # splits of a row-major tensor would produce strided descriptors)
        xt0 = xpool.tile([128, N], F32, tag="x0", name="xt0")
        xt1 = xpool.tile([128, N], F32, tag="x1", name="xt1")
        nc.sync.dma_start(xt0[:, 0:512], io["x0a"])
        nc.scalar.dma_start(xt0[:, 512:1024], io["x0b"])
        nc.gpsimd.dma_start(xt1[:], io["x1"])
        cb_t = cpool.tile([1, O], F32, tag="cb")
        nc.scalar.dma_start(cb_t[:], io["cb"])
        lhs = cpool.tile([4, 2 * M], BF16, tag="lhs")
        nc.scalar.dma_start(lhs[:], io["lhs"])
        pack = cpool.tile([128, PK_COLS], F32, tag="pack")
        nc.scalar.dma_start(pack[:], io["pack"])
        ones_t = cpool.tile([1, B_LOC], F32, tag="ones")
        nc.vector.memset(ones_t[:], 1.0)

        # z columns per branch: [M, B_LOC]
        z_t = [
            cpool.tile([M, B_LOC], F32, tag=f"z{br}", name=f"z{br}")
            for br in range(2)
        ]

        # ---- per x-tensor: d-reduce (max/min), bf16 cast, row shuffle ----
        # x shard viewed as [128, 1024]: partition p = b*16 + n//64,
        # free f = (n%64)*16 + d.
        xt = [xt0, xt1]
        # x0 rows: 2 on sync + 2 on scalar (prep phase, both idle);
        # x1 rows: all on sync (scalar is running sigmoids by then).
        row_eng = [[nc.sync, nc.scalar, nc.sync, nc.scalar],
                   [nc.sync, nc.sync, nc.sync, nc.sync]]
        from concourse.bass import _add_dep_helper

        r_t = []
        last_cast = None
        for xi in range(2):
            x3 = xt[xi][:].rearrange("p (c d) -> p c d", d=D)
            xmax = spool.tile([128, 64], F32, tag=f"xmax{xi}", name="xmax")
            xmin = spool.tile([128, 64], F32, tag=f"xmin{xi}", name="xmin")
            comb = spool.tile([128, 128], BF16, tag=f"comb{xi}", name="comb")
            # DVE stream: red_max, cast_max, red_min, cast_min - each cast
            # unblocks its two row-shuffle DMAs as early as possible
            ops = []
            ops.append(nc.vector.tensor_reduce(xmax[:], x3, axis=AX.X, op=ALU.max))
            ops.append(nc.vector.tensor_copy(comb[:, 0:64], xmax[:]))
            ops.append(nc.vector.tensor_reduce(xmin[:], x3, axis=AX.X, op=ALU.min))
            ops.append(nc.vector.tensor_copy(comb[:, 64:128], xmin[:]))
            if last_cast is not None:
                # keep x1's DVE work behind x0's casts in the static DVE
                # stream (the scheduler otherwise interleaves them and
                # head-of-line-blocks x0's row shuffle for ~2us)
                for op in ops:
                    _add_dep_helper(
                        op.ins, last_cast.ins, sync=False, reason="x0 chain first"
                    )
            last_cast = ops[-1]
            # R rows = xm xm xn xn; free = b*1024 + c*64 + j (n = c*64+j)
            rt = spool.tile([4, B_LOC * N], BF16, tag=f"r{xi}", name="rt")
            for row, src in enumerate([0, 0, 1, 1]):
                row_eng[xi][row].dma_start(
                    rt[row : row + 1, :].rearrange(
                        "p (b c j) -> p b c j", c=16, j=64
                    ),
                    comb[:, src * 64 : (src + 1) * 64],
                )
            r_t.append(rt)

        # ---- branch core: K=4 matmul + fused sigmoid/bias/accum ----
        # lhs rows: wph wpl wnh wnl pair with rt rows: xm xm xn xn
        def unit(br, b):
            lt = lhs[:, br * M : (br + 1) * M]
            vt = pack[:, PK_V0 + br : PK_V0 + br + 1]
            ps = pspool.tile([M, N], F32, tag="s", name="ps")
            rhs = r_t[br][:, b * N : (b + 1) * N]
            nc.tensor.matmul(ps[:, 0:512], lt, rhs[:, 0:512], start=True, stop=True)
            nc.tensor.matmul(
                ps[:, 512:1024], lt, rhs[:, 512:1024], start=True, stop=True
            )
            sg = sigpool.tile([M, N], F32, tag="sg", name="sg")
            nc.scalar.activation(
                sg[:],
                ps[:],
                AF.Sigmoid,
                bias=vt,
                accum_out=z_t[br][:, b : b + 1],
            )

        # ---- MLP head (per batch-half so half 0 hides under sigmoids):
        # h = sigmoid(W_h @ z); y = C_w @ h + C_b ----
        y_half = [
            spool.tile([4, O], F32, tag=f"ysb{h}", name=f"ysb{h}")
            for h in range(2)
        ]

        def mlp_half(half):
            bs = slice(half * 4, half * 4 + 4)
            h_t = []
            for hh in range(2):
                hp = mlppool.tile([128, 4], F32, tag="mlp", name="hp")
                nc.tensor.matmul(
                    hp[:],
                    pack[:, PK_WHT0 + hh * 128 : PK_WHT0 + (hh + 1) * 128],
                    z_t[0][:, bs],
                    start=True,
                    stop=False,
                )
                nc.tensor.matmul(
                    hp[:],
                    pack[:, PK_WHT1 + hh * 128 : PK_WHT1 + (hh + 1) * 128],
                    z_t[1][:, bs],
                    start=False,
                    stop=True,
                )
                hs = spool.tile([128, 4], F32, tag=f"hs{hh}_{half}", name="hs")
                nc.scalar.activation(hs[:], hp[:], AF.Sigmoid)
                h_t.append(hs)
            yp = mlppool.tile([4, O], F32, tag="mlp", name="yp")
            nc.tensor.matmul(
                yp[:], h_t[0][:], pack[:, PK_CW0 : PK_CW0 + O], start=True, stop=False
            )
            nc.tensor.matmul(
                yp[:], h_t[1][:], pack[:, PK_CW1 : PK_CW1 + O], start=False, stop=False
            )
            nc.tensor.matmul(
                yp[:], ones_t[:, 0:4], cb_t[:], start=False, stop=True
            )
            nc.vector.tensor_copy(y_half[half][:], yp[:])

        for b in range(B_LOC):
            unit(0, b)
        for b in range(B_LOC):
            unit(1, b)
            if b == 3:
                mlp_half(0)
        mlp_half(1)
        nc.sync.dma_start(io["y"][0:4, :], y_half[0][:])
        nc.scalar.dma_start(io["y"][4:8, :], y_half[1][:])


_CACHED = None


def _build():
    global _CACHED
    if _CACHED is not None:
        return _CACHED
    nc = bacc.Bacc(
        "TRN2", target_bir_lowering=False, debug=False, num_devices=NCORES
    )
    io = {}
    io["x0a"] = nc.dram_tensor("x0a", [128, 512], F32, kind="ExternalInput").ap()
    io["x0b"] = nc.dram_tensor("x0b", [128, 512], F32, kind="ExternalInput").ap()
    io["x1"] = nc.dram_tensor("x1", [128, N], F32, kind="ExternalInput").ap()
    io["pack"] = nc.dram_tensor(
        "pack", [128, PK_COLS], F32, kind="ExternalInput"
    ).ap()
    io["lhs"] = nc.dram_tensor("lhs", [4, 2 * M], BF16, kind="ExternalInput").ap()
    io["cb"] = nc.dram_tensor("cb", [1, O], F32, kind="ExternalInput").ap()
    io["y"] = nc.dram_tensor("y", [B_LOC, O], F32, kind="ExternalOutput").ap()

    with tile.TileContext(nc) as tc:
        _emit(tc, io)
    nc.compile()
    _CACHED = nc
    return nc


def _prep_params(inputs):
    import ml_dtypes

    f = np.float32
    bf = ml_dtypes.bfloat16

    def branch_lhs(P, U, A):
        W_g = (U @ A).reshape(M, L).astype(np.float64)
        w = (W_g @ P.astype(np.float64))[:, 0]
        rows = []
        for part in (np.maximum(w, 0.0), np.minimum(w, 0.0)):
            hi = part.astype(f).astype(bf)
            lo = (part.astype(f) - hi.astype(f)).astype(bf)
            rows += [hi, lo]
        # rows: wph wpl wnh wnl (pair with R's xm xm xn xn)
        return np.stack(rows).astype(bf)

    pack = np.zeros((128, PK_COLS), dtype=f)
    pack[:, PK_WHT0 : PK_WHT0 + 256] = inputs["W_h"].T[0:128, :]
    pack[:, PK_WHT1 : PK_WHT1 + 256] = inputs["W_h"].T[128:256, :]
    pack[:, PK_CW0 : PK_CW0 + O] = inputs["C_w"].T[0:128, :]
    pack[:, PK_CW1 : PK_CW1 + O] = inputs["C_w"].T[128:256, :]
    pack[:, PK_V0] = inputs["V0"].astype(f)
    pack[:, PK_V1] = inputs["V1"].astype(f)

    lhs = np.concatenate(
        [
            branch_lhs(inputs["P0"], inputs["U0"], inputs["A0"]),
            branch_lhs(inputs["P1"], inputs["U1"], inputs["A1"]),
        ],
        axis=1,
    )

    return {
        "pack": pack,
        "lhs": np.ascontiguousarray(lhs),
        "cb": np.ascontiguousarray(inputs["C_b"].reshape(1, O), dtype=f),
    }


def run(inputs, trace=False, **kw):
    nc = _build()
    params = _prep_params(inputs)
    x0 = np.ascontiguousarray(inputs["x0"], dtype=np.float32)
    x1 = np.ascontiguousarray(inputs["x1"], dtype=np.float32)
    in_maps = []
    for c in range(NCORES):
        m = dict(params)
        xc0 = x0[c * B_LOC : (c + 1) * B_LOC].reshape(128, N)
        m["x0a"] = np.ascontiguousarray(xc0[:, 0:512])
        m["x0b"] = np.ascontiguousarray(xc0[:, 512:1024])
        m["x1"] = x1[c * B_LOC : (c + 1) * B_LOC].reshape(128, N)
        in_maps.append(m)
    res = run_bass_kernel_spmd(nc, in_maps, list(range(NCORES)), trace=trace, **kw)
    y = np.concatenate([res.results[c]["y"] for c in range(NCORES)], axis=0)
    return y, res


def kernel(**inputs):
    y, _ = run(inputs, trace=False)
    return y
